# revision 11
# baseline (speedup 1.0000x reference)
"""CAM (channel attention) kernel for Trainium2, data-parallel over batch.

out[b] = gamma * (a[b] @ softmax(a[b]^T a[b])) + x[b],  a[b] = x[b].reshape(HW, C)

Per core (one batch element):
  Phase A: stream a in [128, 512] double-tiles; cast to bf16 (GpSimd) and
           accumulate aTa = a^T a in PSUM with bf16 matmuls (K=16384; the
           softmax result is insensitive to the mantissa of aTa because the
           diagonal dominates by ~1e4). Build most of aT (c-major layout)
           via f32r PE transpose-mode (value-preserving at f32r precision),
           packed four per PSUM bank, evacuated to SBUF by DVE.
  Softmax: row-softmax of aTa folded into M = gamma * attn + I, so
           out = a @ M (residual + gamma fused into the small matrix).
  Phase B: remaining transposes + out rows = aT_chunk.T @ M (f32r matmuls,
           K=256), evacuated PSUM -> SBUF (ACT) -> DRAM.
Dummy bf16 matmuls warm the PE clock (HAM) at start and keep it warm
across the softmax barrier.
"""

import sys

import numpy as np

for _p in ("/opt/trn_rl_repo",):
    if _p not in sys.path:
        sys.path.insert(0, _p)

import concourse.bass as bass
import concourse.tile as tile
from concourse import bacc, mybir
from concourse.bass_utils import run_bass_kernel_spmd

B, H, W, C = 8, 128, 128, 256
HW = H * W
P = 128
NT = HW // P          # 128 row-tiles of a
ND = NT // 2          # 64 double-tiles
N_CORES = 8

f32 = mybir.dt.float32
f32r = mybir.dt.float32r
bf16 = mybir.dt.bfloat16
ts = bass.ts


def _cam_body(tc, y_out, x_in, g_in):
    nc = tc.nc
    import contextlib

    with contextlib.ExitStack() as ctx:
        const = ctx.enter_context(tc.tile_pool(name="const", bufs=1))
        abig = ctx.enter_context(tc.tile_pool(name="abig", bufs=1))
        aring = ctx.enter_context(tc.tile_pool(name="aring", bufs=8))
        oring = ctx.enter_context(tc.tile_pool(name="oring", bufs=8))
        sm = ctx.enter_context(tc.tile_pool(name="sm", bufs=1))

        # constants: f32r identity + broadcast gamma + bf16 warmup scratch
        ones = const.tile([P, P], f32)
        nc.vector.memset(ones[:], 1.0)
        ident = const.tile([P, P], f32)
        nc.gpsimd.affine_select(
            ident[:], ones[:], pattern=[[1, P]],
            compare_op=mybir.AluOpType.is_equal, fill=0.0,
            base=0, channel_multiplier=-1,
        )
        identr = const.tile([P, P], f32r)
        nc.vector.tensor_copy(identr[:], ident[:])
        warm = const.tile([P, C], bf16)
        nc.vector.memset(warm[:], 0.5)

        g_sb = const.tile([1, 1], f32)
        nc.sync.dma_start(g_sb[0:1, 0:1], g_in[0:1])
        g_bc = const.tile([P, 1], f32)
        nc.gpsimd.partition_broadcast(g_bc[:], g_sb[0:1, :])

        aT_all = abig.tile([P, NT * C], f32r)

        def transposes_for(d, a_dt, tpool):
            """PE-transpose both row-tiles of double-tile d into one packed
            PSUM bank, then evacuate to aT_all (DVE)."""
            tp = tpool.tile([P, 2 * C], f32r, name=f"tp{d}", tag="tp")
            for j in range(2):
                a_i = a_dt[:, j * C:(j + 1) * C]
                for k in range(2):
                    nc.tensor.transpose(
                        tp[:, j * C + k * P: j * C + (k + 1) * P],
                        a_i[:, ts(k, P)], identr[:],
                    )
            i0 = 2 * d
            nc.vector.tensor_copy(aT_all[:, i0 * C:(i0 + 1) * C], tp[:, 0:C])
            nc.scalar.copy(aT_all[:, (i0 + 1) * C:(i0 + 2) * C], tp[:, C:2 * C])

        with tc.tile_pool(name="psD", bufs=1, space="PSUM") as psD:
            # HAM warmup: keep PE busy with dummy bf16 matmuls while the
            # first DMAs land (~4us to flip the clock gate to 2.4 GHz).
            wps = psD.tile([P, C], f32)
            for _ in range(18):
                nc.tensor.matmul(wps[:], warm[:, 0:P], warm[:],
                                 start=True, stop=True)

            with (
                tc.tile_pool(name="psA", bufs=2, space="PSUM") as psA,
                tc.tile_pool(name="psT", bufs=4, space="PSUM") as psT,
            ):
                aTa_ps = [psA.tile([P, C], f32, tag="aTa", name=f"aTa{k}")
                          for k in range(2)]

                # ---- Phase A ----
                for d in range(ND):
                    a_dt = aring.tile([P, 2 * C], f32r, name=f"a{d}", tag="a")
                    nc.sync.dma_start(
                        a_dt[:].rearrange("p (j c) -> p j c", j=2),
                        x_in[ts(d, 2 * P), :].bitcast(f32r).rearrange(
                            "(j p) c -> p j c", p=P
                        ),
                    )
                    for j in range(2):
                        i = 2 * d + j
                        a_i = a_dt[:, j * C:(j + 1) * C]
                        for k in range(2):
                            nc.tensor.matmul(
                                aTa_ps[k][:],
                                a_i[:, ts(k, P)],
                                a_i[:],
                                start=(i == 0),
                                stop=(i == NT - 1),
                                skip_group_check=True,
                            )
                    transposes_for(d, a_dt, psT)

                # ---- Softmax -> M = gamma * attn + I ----
                Ms = []
                for k in range(2):
                    negmx = sm.tile([P, 1], f32, name=f"negmx{k}")
                    nc.vector.tensor_reduce(
                        out=negmx[:], in_=aTa_ps[k][:], op=mybir.AluOpType.max,
                        axis=mybir.AxisListType.X, negate=True,
                    )
                    e = sm.tile([P, C], f32, name=f"e{k}")
                    s = sm.tile([P, 1], f32, name=f"s{k}")
                    nc.scalar.activation(
                        e[:], aTa_ps[k][:], mybir.ActivationFunctionType.Exp,
                        bias=negmx[:, 0:1], scale=1.0, accum_out=s[:],
                    )
                    r = sm.tile([P, 1], f32, name=f"r{k}")
                    nc.vector.reciprocal(r[:], s[:])
                    rg = sm.tile([P, 1], f32, name=f"rg{k}")
                    nc.vector.tensor_mul(rg[:], r[:], g_bc[:])
                    Mk = sm.tile([P, C], f32r, name=f"M{k}")
                    nc.vector.tensor_scalar_mul(Mk[:], e[:], rg[:, 0:1])
                    nc.vector.tensor_add(Mk[:, ts(k, P)], Mk[:, ts(k, P)],
                                         identr[:])
                    Ms.append(Mk)

            # keep PE warm across the softmax barrier
            for _ in range(16):
                nc.tensor.matmul(wps[:], warm[:, 0:P], warm[:],
                                 start=True, stop=True)

        with tc.tile_pool(name="psO", bufs=8, space="PSUM") as psO:
            # ---- Phase B ----
            for d in range(ND):
                o_dt = oring.tile([P, 2 * C], f32, name=f"o{d}", tag="o")
                ops = psO.tile([P, 2 * C], f32, name=f"ops{d}", tag="ops")
                for j in range(2):
                    i = 2 * d + j
                    for k in range(2):
                        nc.tensor.matmul(
                            ops[:, ts(j, C)],
                            aT_all[:, i * C + k * P: i * C + (k + 1) * P],
                            Ms[k][:],
                            start=(k == 0),
                            stop=(k == 1),
                        )
                nc.vector.tensor_copy(o_dt[:, 0:C], ops[:, 0:C])
                nc.scalar.copy(o_dt[:, C:2 * C], ops[:, C:2 * C])
                nc.sync.dma_start(
                    y_out[ts(d, 2 * P), :].rearrange("(j p) c -> p j c", p=P),
                    o_dt[:].rearrange("p (j c) -> p j c", j=2),
                )


_CACHE = {}


def _build():
    nc = bacc.Bacc("TRN2", target_bir_lowering=False, debug=False,
                   num_devices=N_CORES)
    x_in = nc.dram_tensor("x", [HW, C], f32, kind="ExternalInput").ap()
    g_in = nc.dram_tensor("gamma", [1], f32, kind="ExternalInput").ap()
    y_out = nc.dram_tensor("y", [HW, C], f32, kind="ExternalOutput").ap()
    with tile.TileContext(nc) as tc:
        _cam_body(tc, y_out, x_in, g_in)
    nc.compile()
    return nc


def _run(x, gamma, trace=False):
    if "nc" not in _CACHE:
        _CACHE["nc"] = _build()
    nc = _CACHE["nc"]
    xs = np.ascontiguousarray(np.asarray(x, dtype=np.float32).reshape(B, HW, C))
    g = np.ascontiguousarray(np.asarray(gamma, dtype=np.float32).reshape(1))
    in_maps = [{"x": xs[b], "gamma": g} for b in range(B)]
    return run_bass_kernel_spmd(nc, in_maps, core_ids=list(range(N_CORES)),
                                trace=trace)


def kernel(x, gamma):
    res = _run(x, gamma, trace=False)
    out = np.stack([res.results[b]["y"] for b in range(B)], axis=0)
    return out.reshape(B, H, W, C).astype(np.float32)
